# revision 33
# baseline (speedup 1.0000x reference)
"""Causal multi-head attention kernel for 8 Trainium2 NeuronCores.

Problem: B=2, N=2048, C=1024, H=16 heads (hd=64), fp32.
  qkv = x @ w_qkv; per head: S = q k^T * hd^-0.5 (causal),
  out = softmax(S) v; y = out @ w_proj + b_proj.

Sharding (SPMD, one NEFF on 8 cores): core c -> batch b = c // 4,
head group g = c % 4 (heads 4g..4g+3). Column-parallel qkv weights,
row-parallel proj; the host sums the 4 per-core partial projections
per batch and adds the bias (row-parallel unshard combine).

Device-side layout is fully transposed ("feature on partitions"):
  qkvT[f, t] = w_shard^T x^T computed directly by the PE,
  S^T[j, i]  = k^T(lhsT) q^T(rhs), 2 heads row-packed per kj step
  into one 2-bank PSUM strip, causal mask added on the PE,
  P^T = exp(S^T) in a SINGLE ACTIVATE per kj (both heads, 3D AP)
  to amortize the ~352-cycle ACT instruction overhead,
  PV with [v | ones] as lhsT -> rowsum rides in output row 64,
  1/rowsum at 32-lane parallelism via DVE 32x32 stream transposes,
  normalization broadcast via two concurrent K=1 column-tiled matmuls,
  partial = att^T.T @ w_proj over 2 head-pair passes, bf16 output.

All heavy matmul operands are bf16 (fp32 PSUM accumulate): same PE
stream rate as float32r but with fast weight load, half the DMA/SBUF
traffic, and 2x DVE copies. Pass-1 chunks run in reverse (longest
first) with the projection of the previous chunk interleaved into the
attention kj stream so the PE never idles long enough for the HAM
clock gate to re-throttle it to 1.2 GHz.
"""

import os

import numpy as np
import ml_dtypes

import concourse.bass as bass
import concourse.mybir as mybir
import concourse.tile as tile
from concourse import bacc
from concourse.bass_utils import run_bass_kernel_spmd
from concourse.masks import make_causal_mask, make_identity

B, N, C, H = 2, 2048, 1024, 16
HD = C // H  # 64
NCORES = 8
NGROUPS = 4          # head groups (cores per batch)
HPC = H // NGROUPS   # heads per core = 4
KT = C // 128        # 8 contraction tiles
MT = 3 * HPC * HD // 128  # 6 qkvT m-tiles (q0 q1 k0 k1 v0 v1)
F32 = mybir.dt.float32
F32R = mybir.dt.float32r
BF16 = mybir.dt.bfloat16

LAST_RESULTS = None  # BassKernelResults of the most recent run (for test.py)

_NC_CACHE = None


def _build_nc():
    nc = bacc.Bacc("TRN2", target_bir_lowering=False, debug=False,
                   num_devices=NCORES)

    xt_d = nc.dram_tensor("xt", [KT, 128, N], BF16, kind="ExternalInput")
    wqkv_d = nc.dram_tensor("wqkv", [128, MT, KT * 128], BF16,
                            kind="ExternalInput")
    wproj_d = nc.dram_tensor("wproj", [128, 2, C], BF16, kind="ExternalInput")
    part_d = nc.dram_tensor("part", [N, C], BF16, kind="ExternalOutput")

    with tile.TileContext(nc) as tc:
        import contextlib
        ctx = contextlib.ExitStack()
        with ctx:
            consts = ctx.enter_context(tc.tile_pool(name="consts", bufs=1))
            p_xt = ctx.enter_context(tc.tile_pool(name="xt", bufs=33))
            p_qkvT = ctx.enter_context(tc.tile_pool(name="qkvT", bufs=2))
            p_v = ctx.enter_context(tc.tile_pool(name="vall", bufs=2))
            p_P = ctx.enter_context(tc.tile_pool(name="P", bufs=4))
            p_att = ctx.enter_context(tc.tile_pool(name="att", bufs=6))
            p_osb = ctx.enter_context(tc.tile_pool(name="osb", bufs=3))
            p_rc = ctx.enter_context(tc.tile_pool(name="rc", bufs=1))
            p_out = ctx.enter_context(tc.tile_pool(name="out", bufs=3))
            ps_s = ctx.enter_context(
                tc.tile_pool(name="ps_s", bufs=2, space="PSUM"))
            ps_o = ctx.enter_context(
                tc.tile_pool(name="ps_o", bufs=2, space="PSUM"))
            ps_x = ctx.enter_context(
                tc.tile_pool(name="ps_x", bufs=2, space="PSUM"))

            # --- constants ---
            ident_f = consts.tile([128, 128], F32, tag="ident_f")
            make_identity(nc, ident_f[:])
            ident_b = consts.tile([128, 128], BF16, tag="ident_b")
            nc.vector.tensor_copy(out=ident_b[:], in_=ident_f[:])
            # Additive causal mask, applied on the PE as an accumulating
            # matmul: s += trineg.T @ I. trineg[x, y] = 0 if y <= x else
            # -1e30, so (trineg.T)[j, i] = -1e30 where j > i (masked keys).
            trineg_f = consts.tile([128, 128], F32, tag="trineg_f")
            make_causal_mask(nc, trineg_f[:], mask_val=-1e30)
            trineg = consts.tile([128, 128], BF16, tag="trineg")
            nc.vector.tensor_copy(out=trineg[:], in_=trineg_f[:])
            # ones rows for the K=1 1/rowsum broadcast matmuls and the
            # PV rowsum columns (memset cannot write f32r -> stage + cast)
            ones_f = consts.tile([128, 1], F32, tag="ones_f")
            nc.vector.memset(ones_f[:], 1.0)
            ones_b = consts.tile([128, 1], BF16, tag="ones_b")
            nc.vector.tensor_copy(out=ones_b[:], in_=ones_f[:])
            zeros_b = consts.tile([128, 1], BF16, tag="zeros_b")
            nc.vector.memset(zeros_b[:], 0.0)
            zeros_f = consts.tile([128, 1], F32, tag="zeros_f")
            nc.vector.memset(zeros_f[:], 0.0)
            # Complementary K=1 selector rows: broadcast h0's 1/rowsum to
            # partitions 0:64 and h1's to 64:128 of ONE PSUM bank via two
            # accumulating full-width matmuls.
            sel_lo = consts.tile([1, 128], F32R, tag="sel_lo")
            nc.vector.tensor_copy(
                out=sel_lo[0:1, 0:64], in_=ones_f[0:1, :].to_broadcast((1, 64)))
            nc.vector.tensor_copy(
                out=sel_lo[0:1, 64:128],
                in_=zeros_f[0:1, :].to_broadcast((1, 64)))
            sel_hi = consts.tile([1, 128], F32R, tag="sel_hi")
            nc.vector.tensor_copy(
                out=sel_hi[0:1, 0:64],
                in_=zeros_f[0:1, :].to_broadcast((1, 64)))
            nc.vector.tensor_copy(
                out=sel_hi[0:1, 64:128],
                in_=ones_f[0:1, :].to_broadcast((1, 64)))

            # x + wqkv DMAs for BOTH halves up front (33 xt bufs) so the
            # half-1 slices stream in while half-0 computes. Weights are
            # m-major: the q0/k0/v0 m-tiles land first so the first
            # attention chunk can start ~20us earlier.
            wqkv = consts.tile([128, MT, KT * 128], BF16, tag="wqkv")
            xts_all = [[None] * (2 * KT) for _ in range(2)]

            def dma_w_piece(m, k0, nk=2):
                nc.sync.dma_start(wqkv[:, m, k0 * 128:(k0 + nk) * 128],
                                  wqkv_d.ap()[:, m, k0 * 128:(k0 + nk) * 128])

            def dma_x(half, k, cc):
                xk = p_xt.tile([128, 512], BF16, tag="xt")
                c0 = half * 1024 + cc * 512
                nc.sync.dma_start(xk[:], xt_d.ap()[k, :, c0:c0 + 512])
                xts_all[half][2 * k + cc] = xk

            # Need-ordered fine-grained pieces: with the DMA rings draining
            # round-robin, completion time tracks piece size, so the first
            # stage-1 groups' operands (m0/m2 weights + x cc0 slices) are
            # split small and queued first.
            for k in range(0, KT, 2):
                dma_w_piece(0, k)
                dma_x(0, k, 0)
                dma_x(0, k + 1, 0)
                dma_w_piece(2, k)
            for k in range(0, KT, 2):
                dma_w_piece(4, k)
                dma_x(0, k, 1)
                dma_x(0, k + 1, 1)
            for m in (1, 3, 5):
                nc.sync.dma_start(wqkv[:, m, :], wqkv_d.ap()[:, m, :])
            for k in range(KT):
                for cc in range(2):
                    dma_x(1, k, cc)
            wproj = consts.tile([128, 2, C], BF16, tag="wproj")

            qkvT = [None, None]   # per half: [128, MT, 1024] bf16
            v_all = [None, None]  # per pass: [128, 16, 192] bf16
            att_t = [[None] * 4, [None] * 4]  # [pass][ci]

            def stage1_filler(half, groups, on_act):
                """qkvT[:, m, :] = w_m^T @ x^T for token half `half`, over
                the (m, cc) `groups` in order. Yields twice per group
                (after 4 of the 8 accumulating matmuls and after the copy)
                so the attention stream can pace it finely."""
                xts = xts_all[half]
                if qkvT[half] is None:
                    qk_t = p_qkvT.tile([128, MT, 1024], BF16, tag="qkvT")
                    qkvT[half] = qk_t
                else:
                    qk_t = qkvT[half]
                for m, cc in groups:
                    ps = ps_x.tile([128, 512], F32, tag="x")
                    for k in range(KT):
                        nc.tensor.matmul(
                            ps[:],
                            wqkv[:, m, k * 128:(k + 1) * 128],
                            xts[2 * k + cc][:],
                            start=(k == 0), stop=(k == KT - 1))
                        if k == 3:
                            yield
                    dst = qk_t[:, m, cc * 512:(cc + 1) * 512]
                    if on_act:
                        nc.scalar.copy(out=dst, in_=ps[:])
                    else:
                        nc.vector.tensor_copy(out=dst, in_=ps[:])
                    yield

            def stage1(half, groups, on_act=True):
                for _ in stage1_filler(half, groups, on_act):
                    pass

            def v_trans_filler(pss_list, half):
                """Transpose v^T -> v for head pairs in `pss_list`, j-blocks
                of `half`. Layout per j-block: [v_h0(64) | 1 | pad | v_h1(64)
                | 1 | pad]; the ones column puts each head's softmax
                denominator in row 64 of its PV output bank. Yields after
                every 2 transposed blocks."""
                for pss in pss_list:
                    if v_all[pss] is None:
                        va = p_v.tile([128, 16, 192], BF16, tag="vall")
                        v_all[pss] = va
                        nc.vector.tensor_copy(
                            out=va[:, :, 64:65],
                            in_=ones_b[:, None, :].to_broadcast((128, 16, 1)))
                        nc.vector.tensor_copy(
                            out=va[:, :, 65:96],
                            in_=zeros_b[:, None, :].to_broadcast(
                                (128, 16, 31)))
                        nc.vector.tensor_copy(
                            out=va[:, :, 160:161],
                            in_=ones_b[:, None, :].to_broadcast((128, 16, 1)))
                        nc.vector.tensor_copy(
                            out=va[:, :, 161:192],
                            in_=zeros_b[:, None, :].to_broadcast(
                                (128, 16, 31)))
                    va = v_all[pss]
                    for jj in range(8):
                        jb = half * 8 + jj
                        ps = ps_x.tile([128, 128], BF16, tag="x")
                        nc.tensor.transpose(
                            ps[:],
                            qkvT[half][:, 4 + pss, jj * 128:(jj + 1) * 128],
                            ident_b[:])
                        nc.vector.tensor_copy(out=va[:, jb, 0:64],
                                              in_=ps[:, 0:64])
                        nc.vector.tensor_copy(out=va[:, jb, 96:160],
                                              in_=ps[:, 64:128])
                        if jj % 2 == 1:
                            yield

            def v_trans(pss, half):
                for _ in v_trans_filler([pss], half):
                    pass

            def _noop_filler():
                return iter(())

            def attn_chunk(pss, ci, filler=None, drain=True, pre=None,
                           pace=1):
                """One 512-query chunk of attention for head pair `pss`.

                PV is software-pipelined one kj behind the S/exp pair so
                the strictly-FIFO PE queue never parks on an exp wait.
                `filler` emits small batches of independent PE work
                (stage-1 / projection / v-transpose) between kj steps;
                `pre` emits the previous chunk's deferred normalization
                (whose DVE reciprocal chain needs time to drain) two kj
                into this chunk."""
                if filler is None:
                    filler = _noop_filler()
                mq, mk = pss, 2 + pss
                i0 = 512 * ci
                half_q = ci // 2
                iq0 = (i0 % 1024)
                kj_last = 4 * ci + 3
                o0 = ps_o.tile([128, 512], F32, tag="o")
                o1 = ps_o.tile([128, 512], F32, tag="o")
                va = v_all[pss]
                prev = None  # (pt, kj, off) awaiting its PV pair

                def emit_pv(pt, kj, off):
                    nc.tensor.matmul(
                        o0[0:96, off:512], va[:, kj, 0:96],
                        pt[:, 0, off:512],
                        start=(kj == 0), stop=(kj == kj_last))
                    nc.tensor.matmul(
                        o1[0:96, off:512], va[:, kj, 96:192],
                        pt[:, 1, off:512],
                        start=(kj == 0), stop=(kj == kj_last))

                for kj in range(kj_last + 1):
                    off = max(0, kj * 128 - i0)
                    jh = kj // 8
                    jc0 = (kj % 8) * 128
                    masked = kj * 128 >= i0  # block containing the diagonal
                    s = ps_s.tile([128, 2, 512], F32, tag="s")
                    nc.tensor.matmul(
                        s[:, 0, off:512],
                        qkvT[jh][0:64, mk, jc0:jc0 + 128],
                        qkvT[half_q][0:64, mq, iq0 + off:iq0 + 512],
                        start=True, stop=True, tile_position=(0, 0))
                    nc.tensor.matmul(
                        s[:, 1, off:512],
                        qkvT[jh][64:128, mk, jc0:jc0 + 128],
                        qkvT[half_q][64:128, mq, iq0 + off:iq0 + 512],
                        start=True, stop=True, tile_position=(64, 0))
                    # One exp for both heads: 3D AP over the 2-bank strip.
                    pt = p_P.tile([128, 2, 512], BF16, tag="P")
                    nc.scalar.activation(
                        pt[:, :, off:512], s[:, :, off:512],
                        mybir.ActivationFunctionType.Exp)
                    if masked:
                        # Zero P above the diagonal (key j > query c) on the
                        # otherwise-idle GpSimd engine: keeps the PE out of
                        # the mask business and off the exp critical path.
                        for i in range(2):
                            nc.gpsimd.affine_select(
                                out=pt[:, i, off:off + 128],
                                in_=pt[:, i, off:off + 128],
                                compare_op=mybir.AluOpType.is_ge,
                                fill=0.0, base=0, channel_multiplier=-1,
                                pattern=[[1, 128]])
                    if prev is not None:
                        emit_pv(*prev)
                    prev = (pt, kj, off)
                    if kj == 2 and pre is not None:
                        pre()
                    if kj >= 1:
                        for _ in range(pace):
                            next(filler, None)
                emit_pv(*prev)
                # drain any leftover filler work (short chunks have fewer
                # kj steps than the filler has batches)
                if drain:
                    for _ in filler:
                        pass
                # Quick copies release the PSUM o banks; the DVE reciprocal
                # chain starts right away, and only the PE part of the
                # normalization is deferred into the next chunk (finisher),
                # by which time rc is long done.
                ob0 = p_osb.tile([128, 512], F32, tag="osb")
                ob1 = p_osb.tile([128, 512], F32, tag="osb")
                nc.scalar.copy(out=ob0[0:96, :], in_=o0[0:96, :])
                nc.vector.tensor_copy(out=ob1[0:96, :], in_=o1[0:96, :])
                # Batched reciprocal at 32-lane parallelism: 32x32
                # stream-transpose brings the rowsum row (row 64) onto
                # partitions; reciprocal runs on the strided view (one
                # column per 32-block); transposing back yields the
                # 1/rowsum row for both heads at partition 0.
                tr = p_rc.tile([32, 1024], F32, tag="tr")
                nc.vector.transpose(tr[:, 0:512], ob0[64:96, :])
                nc.vector.transpose(tr[:, 512:1024], ob1[64:96, :])
                rcc = p_rc.tile([32, 1024], F32, tag="rcc")
                nc.vector.reciprocal(rcc[:, 0:1024:32],
                                     tr[:, 0:1024:32])
                rcb = p_rc.tile([32, 1024], F32, tag="rcb")
                nc.vector.transpose(rcb[:], rcc[:])
                rc = p_rc.tile([32, 1024], F32R, tag="rcr")
                with nc.allow_low_precision(
                        reason="1/rowsum feeds an fp32r matmul"):
                    nc.vector.tensor_copy(out=rc[0:1, :],
                                          in_=rcb[0:1, :])

                def finisher():
                    # Broadcast 1/rowsum: h0 -> rows 0:64, h1 -> rows
                    # 64:128 of one PSUM bank via complementary selectors.
                    ps_b = ps_x.tile([128, 512], F32, tag="x")
                    nc.tensor.matmul(ps_b[:], sel_lo[0:1, :],
                                     rc[0:1, 0:512], start=True, stop=False)
                    nc.tensor.matmul(ps_b[:], sel_hi[0:1, :],
                                     rc[0:1, 512:1024], start=False,
                                     stop=True)
                    att = p_att.tile([128, 512], BF16, tag="att")
                    with nc.allow_low_precision(
                            reason="att feeds bf16 matmul"):
                        nc.vector.tensor_mul(att[0:64, :], ob0[0:64, :],
                                             ps_b[0:64, :])
                        nc.vector.tensor_mul(att[64:128, :], ob1[0:64, :],
                                             ps_b[64:128, :])
                    att_t[pss][ci] = att

                return finisher

            def proj_filler(ci):
                """Yield after each small batch of proj work for chunk ci:
                partial[i0:i0+512, :] = att^T.T @ w_proj (both passes)."""
                i0 = 512 * ci
                for tt in range(4):
                    for ec in range(2):
                        ps = ps_x.tile([128, 512], F32, tag="x")
                        nc.tensor.matmul(
                            ps[:],
                            att_t[0][ci][:, tt * 128:(tt + 1) * 128],
                            wproj[:, 0, ec * 512:(ec + 1) * 512],
                            start=True, stop=False)
                        nc.tensor.matmul(
                            ps[:],
                            att_t[1][ci][:, tt * 128:(tt + 1) * 128],
                            wproj[:, 1, ec * 512:(ec + 1) * 512],
                            start=False, stop=True)
                        osb = p_out.tile([128, 512], BF16, tag="out")
                        nc.vector.tensor_copy(out=osb[:], in_=ps[:])
                        nc.sync.dma_start(
                            part_d.ap()[i0 + tt * 128:i0 + (tt + 1) * 128,
                                        ec * 512:(ec + 1) * 512],
                            osb[:])
                        yield

            def proj_now(ci):
                f = proj_filler(ci)
                for _ in range(8):
                    next(f)

            # --- emission order ---
            # Only the m-tiles the first chunk needs (q0/k0/v0) run before
            # attention starts; everything else — rest of stage 1, the v
            # transposes, and each finished chunk pair's projection — flows
            # through one global work queue consumed as PE filler between
            # kj steps of the ACT-bound attention streams. Each chunk's
            # deferred normalization (`pre`) lands one kj into the next.
            import itertools
            stage1(0, [(0, 0), (2, 0), (4, 0), (0, 1), (2, 1), (4, 1)],
                   on_act=True)
            v_trans(0, 0)
            workq = itertools.chain(
                stage1_filler(0, [(1, 0), (1, 1), (3, 0), (3, 1),
                                  (5, 0), (5, 1)], on_act=False),  # 12
                v_trans_filler([1], 0),                            # 4
                stage1_filler(1, [(m, cc) for m in range(MT)
                                  for cc in (0, 1)], on_act=False),  # 24
                v_trans_filler([0, 1], 1),                         # 8
            )
            f01 = attn_chunk(0, 1, filler=workq, drain=False, pace=2)
            f00 = attn_chunk(0, 0, filler=workq, drain=False, pace=2,
                             pre=f01)
            f10 = attn_chunk(1, 0, filler=workq, drain=False, pace=2,
                             pre=f00)
            f11 = attn_chunk(1, 1, filler=workq, drain=False, pace=2,
                             pre=f10)
            # wproj DMA deferred here: keeps it off the critical startup
            # DMA queue (first consumer is proj(0), close by).
            nc.sync.dma_start(wproj[:], wproj_d.ap())
            workq2 = itertools.chain(workq, proj_filler(0), proj_filler(1))
            f02 = attn_chunk(0, 2, filler=workq2, drain=False, pre=f11)
            f12 = attn_chunk(1, 2, filler=workq2, drain=False, pre=f02)
            workq3 = itertools.chain(workq2, proj_filler(2))
            f03 = attn_chunk(0, 3, filler=workq3, drain=False, pre=f12)
            f13 = attn_chunk(1, 3, filler=workq3, pre=f03)
            f13()
            proj_now(3)

    nc.compile()
    return nc


def _get_nc():
    global _NC_CACHE
    if _NC_CACHE is None:
        _NC_CACHE = _build_nc()
    return _NC_CACHE


def _shards(x, w_qkv, w_proj):
    """Build the per-core input maps (host-side sharding)."""
    x = np.asarray(x, np.float32)
    w_qkv = np.asarray(w_qkv, np.float32)
    w_proj = np.asarray(w_proj, np.float32)
    scale = float(HD) ** -0.5

    # xt[b][k, p, n] = x[b, n, 128k + p]
    xts = [np.ascontiguousarray(
        x[b].T.reshape(KT, 128, N)).astype(ml_dtypes.bfloat16)
        for b in range(B)]

    in_maps = []
    for c in range(NCORES):
        b, g = divmod(c, NGROUPS)
        cols = []
        for s in range(3):  # q, k, v
            for hh in range(HPC):
                h = HPC * g + hh
                blk = w_qkv[:, s * C + h * HD: s * C + (h + 1) * HD]
                if s == 0:
                    blk = blk * scale
                cols.append(blk)
        wq = np.ascontiguousarray(
            np.concatenate(cols, axis=1).reshape(KT, 128, MT, 128)
            .transpose(1, 2, 0, 3).reshape(128, MT, KT * 128)
        ).astype(ml_dtypes.bfloat16)
        wp = np.ascontiguousarray(
            w_proj[256 * g:256 * (g + 1), :].reshape(2, 128, C)
            .transpose(1, 0, 2)).astype(ml_dtypes.bfloat16)
        in_maps.append({"xt": xts[b], "wqkv": wq, "wproj": wp})
    return in_maps


def kernel(x, w_qkv, w_proj, b_proj):
    global LAST_RESULTS
    in_maps = _shards(x, w_qkv, w_proj)
    nc = _get_nc()
    trace = os.environ.get("BASS_KERNEL_TRACE", "0") == "1"
    res = run_bass_kernel_spmd(nc, in_maps, core_ids=list(range(NCORES)),
                               trace=trace)
    LAST_RESULTS = res
    b_proj = np.asarray(b_proj, np.float32)
    out = np.empty((B, N, C), np.float32)
    for b in range(B):
        acc = res.results[NGROUPS * b]["part"].astype(np.float64)
        for g in range(1, NGROUPS):
            acc = acc + res.results[NGROUPS * b + g]["part"].astype(
                np.float64)
        out[b] = (acc + b_proj).astype(np.float32)
    return out


# revision 35
# speedup vs baseline: 1.0027x; 1.0027x over previous
"""Causal multi-head attention kernel for 8 Trainium2 NeuronCores.

Problem: B=2, N=2048, C=1024, H=16 heads (hd=64), fp32.
  qkv = x @ w_qkv; per head: S = q k^T * hd^-0.5 (causal),
  out = softmax(S) v; y = out @ w_proj + b_proj.

Sharding (SPMD, one NEFF on 8 cores): core c -> batch b = c // 4,
head group g = c % 4 (heads 4g..4g+3). Column-parallel qkv weights,
row-parallel proj; the host sums the 4 per-core partial projections
per batch and adds the bias (row-parallel unshard combine).

Device-side layout is fully transposed ("feature on partitions"):
  qkvT[f, t] = w_shard^T x^T computed directly by the PE,
  S^T[j, i]  = k^T(lhsT) q^T(rhs), 2 heads row-packed per kj step
  into one 2-bank PSUM strip, causal mask added on the PE,
  P^T = exp(S^T) in a SINGLE ACTIVATE per kj (both heads, 3D AP)
  to amortize the ~352-cycle ACT instruction overhead,
  PV with [v | ones] as lhsT -> rowsum rides in output row 64,
  1/rowsum at 32-lane parallelism via DVE 32x32 stream transposes,
  normalization broadcast via two concurrent K=1 column-tiled matmuls,
  partial = att^T.T @ w_proj over 2 head-pair passes, bf16 output.

All heavy matmul operands are bf16 (fp32 PSUM accumulate): same PE
stream rate as float32r but with fast weight load, half the DMA/SBUF
traffic, and 2x DVE copies. Pass-1 chunks run in reverse (longest
first) with the projection of the previous chunk interleaved into the
attention kj stream so the PE never idles long enough for the HAM
clock gate to re-throttle it to 1.2 GHz.
"""

import os

import numpy as np
import ml_dtypes

import concourse.bass as bass
import concourse.mybir as mybir
import concourse.tile as tile
from concourse import bacc
from concourse.bass_utils import run_bass_kernel_spmd
from concourse.masks import make_causal_mask, make_identity

B, N, C, H = 2, 2048, 1024, 16
HD = C // H  # 64
NCORES = 8
NGROUPS = 4          # head groups (cores per batch)
HPC = H // NGROUPS   # heads per core = 4
KT = C // 128        # 8 contraction tiles
MT = 3 * HPC * HD // 128  # 6 qkvT m-tiles (q0 q1 k0 k1 v0 v1)
F32 = mybir.dt.float32
F32R = mybir.dt.float32r
BF16 = mybir.dt.bfloat16

LAST_RESULTS = None  # BassKernelResults of the most recent run (for test.py)

_NC_CACHE = None


def _build_nc():
    nc = bacc.Bacc("TRN2", target_bir_lowering=False, debug=False,
                   num_devices=NCORES)

    xt_d = nc.dram_tensor("xt", [KT, 128, N], BF16, kind="ExternalInput")
    wqkv_d = nc.dram_tensor("wqkv", [128, MT, KT * 128], BF16,
                            kind="ExternalInput")
    wproj_d = nc.dram_tensor("wproj", [128, 2, C], BF16, kind="ExternalInput")
    part_d = nc.dram_tensor("part", [N, C], BF16, kind="ExternalOutput")

    with tile.TileContext(nc) as tc:
        import contextlib
        ctx = contextlib.ExitStack()
        with ctx:
            consts = ctx.enter_context(tc.tile_pool(name="consts", bufs=1))
            p_xt = ctx.enter_context(tc.tile_pool(name="xt", bufs=33))
            p_qkvT = ctx.enter_context(tc.tile_pool(name="qkvT", bufs=2))
            p_v = ctx.enter_context(tc.tile_pool(name="vall", bufs=2))
            p_P = ctx.enter_context(tc.tile_pool(name="P", bufs=4))
            p_att = ctx.enter_context(tc.tile_pool(name="att", bufs=6))
            p_osb = ctx.enter_context(tc.tile_pool(name="osb", bufs=3))
            p_rc = ctx.enter_context(tc.tile_pool(name="rc", bufs=1))
            p_out = ctx.enter_context(tc.tile_pool(name="out", bufs=3))
            ps_s = ctx.enter_context(
                tc.tile_pool(name="ps_s", bufs=2, space="PSUM"))
            ps_o = ctx.enter_context(
                tc.tile_pool(name="ps_o", bufs=2, space="PSUM"))
            ps_x = ctx.enter_context(
                tc.tile_pool(name="ps_x", bufs=2, space="PSUM"))

            # --- constants ---
            ident_f = consts.tile([128, 128], F32, tag="ident_f")
            make_identity(nc, ident_f[:])
            ident_b = consts.tile([128, 128], BF16, tag="ident_b")
            nc.vector.tensor_copy(out=ident_b[:], in_=ident_f[:])
            # Additive causal mask, applied on the PE as an accumulating
            # matmul: s += trineg.T @ I. trineg[x, y] = 0 if y <= x else
            # -1e30, so (trineg.T)[j, i] = -1e30 where j > i (masked keys).
            trineg_f = consts.tile([128, 128], F32, tag="trineg_f")
            make_causal_mask(nc, trineg_f[:], mask_val=-1e30)
            trineg = consts.tile([128, 128], BF16, tag="trineg")
            nc.vector.tensor_copy(out=trineg[:], in_=trineg_f[:])
            # ones rows for the K=1 1/rowsum broadcast matmuls and the
            # PV rowsum columns (memset cannot write f32r -> stage + cast)
            ones_f = consts.tile([128, 1], F32, tag="ones_f")
            nc.vector.memset(ones_f[:], 1.0)
            ones_b = consts.tile([128, 1], BF16, tag="ones_b")
            nc.vector.tensor_copy(out=ones_b[:], in_=ones_f[:])
            zeros_b = consts.tile([128, 1], BF16, tag="zeros_b")
            nc.vector.memset(zeros_b[:], 0.0)
            zeros_f = consts.tile([128, 1], F32, tag="zeros_f")
            nc.vector.memset(zeros_f[:], 0.0)
            # Complementary K=1 selector rows: broadcast h0's 1/rowsum to
            # partitions 0:64 and h1's to 64:128 of ONE PSUM bank via two
            # accumulating full-width matmuls.
            sel_lo = consts.tile([1, 128], F32R, tag="sel_lo")
            nc.vector.tensor_copy(
                out=sel_lo[0:1, 0:64], in_=ones_f[0:1, :].to_broadcast((1, 64)))
            nc.vector.tensor_copy(
                out=sel_lo[0:1, 64:128],
                in_=zeros_f[0:1, :].to_broadcast((1, 64)))
            sel_hi = consts.tile([1, 128], F32R, tag="sel_hi")
            nc.vector.tensor_copy(
                out=sel_hi[0:1, 0:64],
                in_=zeros_f[0:1, :].to_broadcast((1, 64)))
            nc.vector.tensor_copy(
                out=sel_hi[0:1, 64:128],
                in_=ones_f[0:1, :].to_broadcast((1, 64)))

            # x + wqkv DMAs for BOTH halves up front (33 xt bufs) so the
            # half-1 slices stream in while half-0 computes. Weights are
            # m-major: the q0/k0/v0 m-tiles land first so the first
            # attention chunk can start ~20us earlier.
            wqkv = consts.tile([128, MT, KT * 128], BF16, tag="wqkv")
            xts_all = [[None] * (2 * KT) for _ in range(2)]

            def dma_w_piece(m, k0, nk=2):
                nc.sync.dma_start(wqkv[:, m, k0 * 128:(k0 + nk) * 128],
                                  wqkv_d.ap()[:, m, k0 * 128:(k0 + nk) * 128])

            def dma_x(half, k, cc):
                xk = p_xt.tile([128, 512], BF16, tag="xt")
                c0 = half * 1024 + cc * 512
                nc.sync.dma_start(xk[:], xt_d.ap()[k, :, c0:c0 + 512])
                xts_all[half][2 * k + cc] = xk

            # Need-ordered fine-grained pieces: with the DMA rings draining
            # round-robin, completion time tracks piece size, so the first
            # stage-1 groups' operands (m0/m2 weights + x cc0 slices) are
            # split small and queued first.
            for k in range(0, KT, 2):
                dma_w_piece(0, k)
                dma_x(0, k, 0)
                dma_x(0, k + 1, 0)
                dma_w_piece(2, k)
            for k in range(0, KT, 2):
                dma_w_piece(4, k)
                dma_x(0, k, 1)
                dma_x(0, k + 1, 1)
            for m in (1, 3, 5):
                nc.sync.dma_start(wqkv[:, m, :], wqkv_d.ap()[:, m, :])
            for k in range(KT):
                for cc in range(2):
                    dma_x(1, k, cc)
            wproj = consts.tile([128, 2, C], BF16, tag="wproj")

            qkvT = [None, None]   # per half: [128, MT, 1024] bf16
            v_all = [None, None]  # per pass: [128, 16, 192] bf16
            att_t = [[None] * 4, [None] * 4]  # [pass][ci]

            def stage1_filler(half, groups, on_act):
                """qkvT[:, m, :] = w_m^T @ x^T for token half `half`, over
                the (m, cc) `groups` in order. Yields twice per group
                (after 4 of the 8 accumulating matmuls and after the copy)
                so the attention stream can pace it finely."""
                xts = xts_all[half]
                if qkvT[half] is None:
                    qk_t = p_qkvT.tile([128, MT, 1024], BF16, tag="qkvT")
                    qkvT[half] = qk_t
                else:
                    qk_t = qkvT[half]
                for m, cc in groups:
                    ps = ps_x.tile([128, 512], F32, tag="x")
                    for k in range(KT):
                        nc.tensor.matmul(
                            ps[:],
                            wqkv[:, m, k * 128:(k + 1) * 128],
                            xts[2 * k + cc][:],
                            start=(k == 0), stop=(k == KT - 1))
                        if k == 3:
                            yield
                    dst = qk_t[:, m, cc * 512:(cc + 1) * 512]
                    if on_act:
                        nc.scalar.copy(out=dst, in_=ps[:])
                    else:
                        nc.vector.tensor_copy(out=dst, in_=ps[:])
                    yield

            def stage1(half, groups, on_act=True):
                for _ in stage1_filler(half, groups, on_act):
                    pass

            def v_trans_filler(pss_list, half):
                """Transpose v^T -> v for head pairs in `pss_list`, j-blocks
                of `half`. Layout per j-block: [v_h0(64) | 1 | pad | v_h1(64)
                | 1 | pad]; the ones column puts each head's softmax
                denominator in row 64 of its PV output bank. Yields after
                every 2 transposed blocks."""
                for pss in pss_list:
                    if v_all[pss] is None:
                        va = p_v.tile([128, 16, 192], BF16, tag="vall")
                        v_all[pss] = va
                        nc.vector.tensor_copy(
                            out=va[:, :, 64:65],
                            in_=ones_b[:, None, :].to_broadcast((128, 16, 1)))
                        nc.vector.tensor_copy(
                            out=va[:, :, 65:96],
                            in_=zeros_b[:, None, :].to_broadcast(
                                (128, 16, 31)))
                        nc.vector.tensor_copy(
                            out=va[:, :, 160:161],
                            in_=ones_b[:, None, :].to_broadcast((128, 16, 1)))
                        nc.vector.tensor_copy(
                            out=va[:, :, 161:192],
                            in_=zeros_b[:, None, :].to_broadcast(
                                (128, 16, 31)))
                    va = v_all[pss]
                    for jj in range(8):
                        jb = half * 8 + jj
                        ps = ps_x.tile([128, 128], BF16, tag="x")
                        nc.tensor.transpose(
                            ps[:],
                            qkvT[half][:, 4 + pss, jj * 128:(jj + 1) * 128],
                            ident_b[:])
                        nc.vector.tensor_copy(out=va[:, jb, 0:64],
                                              in_=ps[:, 0:64])
                        nc.vector.tensor_copy(out=va[:, jb, 96:160],
                                              in_=ps[:, 64:128])
                        if jj % 2 == 1:
                            yield

            def v_trans(pss, half):
                for _ in v_trans_filler([pss], half):
                    pass

            def _noop_filler():
                return iter(())

            def attn_chunk(pss, ci, filler=None, drain=True, pre=None,
                           pace=1):
                """One 512-query chunk of attention for head pair `pss`.

                PV is software-pipelined one kj behind the S/exp pair so
                the strictly-FIFO PE queue never parks on an exp wait.
                `filler` emits small batches of independent PE work
                (stage-1 / projection / v-transpose) between kj steps;
                `pre` emits the previous chunk's deferred normalization
                (whose DVE reciprocal chain needs time to drain) two kj
                into this chunk."""
                if filler is None:
                    filler = _noop_filler()
                mq, mk = pss, 2 + pss
                i0 = 512 * ci
                half_q = ci // 2
                iq0 = (i0 % 1024)
                kj_last = 4 * ci + 3
                o0 = ps_o.tile([128, 512], F32, tag="o")
                o1 = ps_o.tile([128, 512], F32, tag="o")
                va = v_all[pss]
                prev = None  # (pt, kj, off) awaiting its PV pair

                def emit_pv(pt, kj, off):
                    nc.tensor.matmul(
                        o0[0:96, off:512], va[:, kj, 0:96],
                        pt[:, 0, off:512],
                        start=(kj == 0), stop=(kj == kj_last))
                    nc.tensor.matmul(
                        o1[0:96, off:512], va[:, kj, 96:192],
                        pt[:, 1, off:512],
                        start=(kj == 0), stop=(kj == kj_last))

                for kj in range(kj_last + 1):
                    off = max(0, kj * 128 - i0)
                    jh = kj // 8
                    jc0 = (kj % 8) * 128
                    masked = kj * 128 >= i0  # block containing the diagonal
                    s = ps_s.tile([128, 2, 512], F32, tag="s")
                    nc.tensor.matmul(
                        s[:, 0, off:512],
                        qkvT[jh][0:64, mk, jc0:jc0 + 128],
                        qkvT[half_q][0:64, mq, iq0 + off:iq0 + 512],
                        start=True, stop=True, tile_position=(0, 0))
                    nc.tensor.matmul(
                        s[:, 1, off:512],
                        qkvT[jh][64:128, mk, jc0:jc0 + 128],
                        qkvT[half_q][64:128, mq, iq0 + off:iq0 + 512],
                        start=True, stop=True, tile_position=(64, 0))
                    # One exp for both heads: 3D AP over the 2-bank strip.
                    pt = p_P.tile([128, 2, 512], BF16, tag="P")
                    nc.scalar.activation(
                        pt[:, :, off:512], s[:, :, off:512],
                        mybir.ActivationFunctionType.Exp)
                    if masked:
                        # Zero P above the diagonal (key j > query c) on the
                        # otherwise-idle GpSimd engine: keeps the PE out of
                        # the mask business and off the exp critical path.
                        for i in range(2):
                            nc.gpsimd.affine_select(
                                out=pt[:, i, off:off + 128],
                                in_=pt[:, i, off:off + 128],
                                compare_op=mybir.AluOpType.is_ge,
                                fill=0.0, base=0, channel_multiplier=-1,
                                pattern=[[1, 128]])
                    if prev is not None:
                        emit_pv(*prev)
                    prev = (pt, kj, off)
                    if kj == 1 and pre is not None:
                        pre()
                    if kj >= 1:
                        for _ in range(pace):
                            next(filler, None)
                emit_pv(*prev)
                # drain any leftover filler work (short chunks have fewer
                # kj steps than the filler has batches)
                if drain:
                    for _ in filler:
                        pass
                # Quick copies release the PSUM o banks; the DVE reciprocal
                # chain starts right away, and only the PE part of the
                # normalization is deferred into the next chunk (finisher),
                # by which time rc is long done.
                ob0 = p_osb.tile([128, 512], F32, tag="osb")
                ob1 = p_osb.tile([128, 512], F32, tag="osb")
                nc.scalar.copy(out=ob0[0:96, :], in_=o0[0:96, :])
                nc.vector.tensor_copy(out=ob1[0:96, :], in_=o1[0:96, :])
                # Batched reciprocal at 32-lane parallelism: 32x32
                # stream-transpose brings the rowsum row (row 64) onto
                # partitions; reciprocal runs on the strided view (one
                # column per 32-block); transposing back yields the
                # 1/rowsum row for both heads at partition 0.
                tr = p_rc.tile([32, 1024], F32, tag="tr")
                nc.vector.transpose(tr[:, 0:512], ob0[64:96, :])
                nc.vector.transpose(tr[:, 512:1024], ob1[64:96, :])
                rcc = p_rc.tile([32, 1024], F32, tag="rcc")
                nc.vector.reciprocal(rcc[:, 0:1024:32],
                                     tr[:, 0:1024:32])
                rcb = p_rc.tile([32, 1024], F32, tag="rcb")
                nc.vector.transpose(rcb[:], rcc[:])
                rc = p_rc.tile([32, 1024], F32R, tag="rcr")
                with nc.allow_low_precision(
                        reason="1/rowsum feeds an fp32r matmul"):
                    nc.vector.tensor_copy(out=rc[0:1, :],
                                          in_=rcb[0:1, :])

                def finisher():
                    # Broadcast 1/rowsum: h0 -> rows 0:64, h1 -> rows
                    # 64:128 of one PSUM bank via complementary selectors.
                    ps_b = ps_x.tile([128, 512], F32, tag="x")
                    nc.tensor.matmul(ps_b[:], sel_lo[0:1, :],
                                     rc[0:1, 0:512], start=True, stop=False)
                    nc.tensor.matmul(ps_b[:], sel_hi[0:1, :],
                                     rc[0:1, 512:1024], start=False,
                                     stop=True)
                    att = p_att.tile([128, 512], BF16, tag="att")
                    with nc.allow_low_precision(
                            reason="att feeds bf16 matmul"):
                        nc.vector.tensor_mul(att[0:64, :], ob0[0:64, :],
                                             ps_b[0:64, :])
                        nc.vector.tensor_mul(att[64:128, :], ob1[0:64, :],
                                             ps_b[64:128, :])
                    att_t[pss][ci] = att

                return finisher

            def proj_filler(ci):
                """Yield after each small batch of proj work for chunk ci:
                partial[i0:i0+512, :] = att^T.T @ w_proj (both passes)."""
                i0 = 512 * ci
                for tt in range(4):
                    for ec in range(2):
                        ps = ps_x.tile([128, 512], F32, tag="x")
                        nc.tensor.matmul(
                            ps[:],
                            att_t[0][ci][:, tt * 128:(tt + 1) * 128],
                            wproj[:, 0, ec * 512:(ec + 1) * 512],
                            start=True, stop=False)
                        nc.tensor.matmul(
                            ps[:],
                            att_t[1][ci][:, tt * 128:(tt + 1) * 128],
                            wproj[:, 1, ec * 512:(ec + 1) * 512],
                            start=False, stop=True)
                        osb = p_out.tile([128, 512], BF16, tag="out")
                        nc.vector.tensor_copy(out=osb[:], in_=ps[:])
                        nc.sync.dma_start(
                            part_d.ap()[i0 + tt * 128:i0 + (tt + 1) * 128,
                                        ec * 512:(ec + 1) * 512],
                            osb[:])
                        yield

            def proj_now(ci):
                f = proj_filler(ci)
                for _ in range(8):
                    next(f)

            # --- emission order ---
            # Only the m-tiles the first chunk needs (q0/k0/v0) run before
            # attention starts; everything else — rest of stage 1, the v
            # transposes, and each finished chunk pair's projection — flows
            # through one global work queue consumed as PE filler between
            # kj steps of the ACT-bound attention streams. Each chunk's
            # deferred normalization (`pre`) lands one kj into the next.
            import itertools
            # HAM warm-up: ~45 free matmuls on the identity while the
            # input DMAs stream in. The PE's clock gate only opens to
            # 2.4 GHz after ~3.4us of sustained activity; these make the
            # otherwise DMA-idle start count toward it.
            ps_w = ps_s.tile([128, 2, 512], F32, tag="s")
            for w in range(45):
                nc.tensor.matmul(ps_w[:, w % 2, 0:128], ident_b[:],
                                 ident_b[:], start=True, stop=True)
            # Minimal prefix for the first attention chunk: the cc0 halves
            # of q0/k0/v0 and the first four v-transpose blocks.
            stage1(0, [(0, 0), (2, 0), (4, 0)], on_act=True)
            vt00 = v_trans_filler([0], 0)
            next(vt00), next(vt00)
            workq = itertools.chain(
                stage1_filler(0, [(0, 1), (2, 1), (4, 1)], on_act=True),
                vt00,
                stage1_filler(0, [(1, 0), (1, 1), (3, 0), (3, 1),
                                  (5, 0), (5, 1)], on_act=False),  # 12
                v_trans_filler([1], 0),                            # 4
                stage1_filler(1, [(m, cc) for m in range(MT)
                                  for cc in (0, 1)], on_act=False),  # 24
                v_trans_filler([0, 1], 1),                         # 8
            )
            f00 = attn_chunk(0, 0, filler=workq, drain=False, pace=3)
            f01 = attn_chunk(0, 1, filler=workq, drain=False, pace=3,
                             pre=f00)
            f10 = attn_chunk(1, 0, filler=workq, drain=False, pace=3,
                             pre=f01)
            f11 = attn_chunk(1, 1, filler=workq, drain=False, pace=3,
                             pre=f10)
            # wproj DMA deferred here: keeps it off the critical startup
            # DMA queue (first consumer is proj(0), close by).
            nc.sync.dma_start(wproj[:], wproj_d.ap())
            workq2 = itertools.chain(workq, proj_filler(0), proj_filler(1))
            f02 = attn_chunk(0, 2, filler=workq2, drain=False, pre=f11)
            f12 = attn_chunk(1, 2, filler=workq2, drain=False, pre=f02)
            workq3 = itertools.chain(workq2, proj_filler(2))
            f03 = attn_chunk(0, 3, filler=itertools.islice(workq3, 8),
                             drain=False, pre=f12)
            f13 = attn_chunk(1, 3, filler=workq3, pre=f03)
            f13()
            proj_now(3)

    nc.compile()
    return nc


def _get_nc():
    global _NC_CACHE
    if _NC_CACHE is None:
        _NC_CACHE = _build_nc()
    return _NC_CACHE


def _shards(x, w_qkv, w_proj):
    """Build the per-core input maps (host-side sharding)."""
    x = np.asarray(x, np.float32)
    w_qkv = np.asarray(w_qkv, np.float32)
    w_proj = np.asarray(w_proj, np.float32)
    scale = float(HD) ** -0.5

    # xt[b][k, p, n] = x[b, n, 128k + p]
    xts = [np.ascontiguousarray(
        x[b].T.reshape(KT, 128, N)).astype(ml_dtypes.bfloat16)
        for b in range(B)]

    in_maps = []
    for c in range(NCORES):
        b, g = divmod(c, NGROUPS)
        cols = []
        for s in range(3):  # q, k, v
            for hh in range(HPC):
                h = HPC * g + hh
                blk = w_qkv[:, s * C + h * HD: s * C + (h + 1) * HD]
                if s == 0:
                    blk = blk * scale
                cols.append(blk)
        wq = np.ascontiguousarray(
            np.concatenate(cols, axis=1).reshape(KT, 128, MT, 128)
            .transpose(1, 2, 0, 3).reshape(128, MT, KT * 128)
        ).astype(ml_dtypes.bfloat16)
        wp = np.ascontiguousarray(
            w_proj[256 * g:256 * (g + 1), :].reshape(2, 128, C)
            .transpose(1, 0, 2)).astype(ml_dtypes.bfloat16)
        in_maps.append({"xt": xts[b], "wqkv": wq, "wproj": wp})
    return in_maps


def kernel(x, w_qkv, w_proj, b_proj):
    global LAST_RESULTS
    in_maps = _shards(x, w_qkv, w_proj)
    nc = _get_nc()
    trace = os.environ.get("BASS_KERNEL_TRACE", "0") == "1"
    res = run_bass_kernel_spmd(nc, in_maps, core_ids=list(range(NCORES)),
                               trace=trace)
    LAST_RESULTS = res
    b_proj = np.asarray(b_proj, np.float32)
    out = np.empty((B, N, C), np.float32)
    for b in range(B):
        acc = res.results[NGROUPS * b]["part"].astype(np.float64)
        for g in range(1, NGROUPS):
            acc = acc + res.results[NGROUPS * b + g]["part"].astype(
                np.float64)
        out[b] = (acc + b_proj).astype(np.float32)
    return out
